# revision 1
# baseline (speedup 1.0000x reference)
"""TRN2 Bass kernel for nn_ST_model_58815282151899 (dense ST-transformer).

Sharding: data-parallel over batch (B=16 -> 2 per core x 8 cores, no collectives).

Key structure (vs naive):
  * Chebyshev collapse: sum_{k<4} T_k(A) = 4A^3 + 2A^2 - 2A =: M (per support).
    M~ = [M1 M2] is precomputed ONCE on device and kept SBUF-resident (bf16).
    Per layer the GNN is then   G = relu(M1 (x Wg1) + M2 (x Wg2) + bg)
    = one 2048-deep PSUM-accumulated matmul (feature transform applied first).
  * Layouts: feature-major fm = [(j=t%2, d) part, tp=t//2, n] for all linears
    and both layernorms (PE ones-matmul stats); node-major nm = [n%128 part,
    (t, d) free] only for the attention core (DVE broadcast ops).
  * The cheb matmul uses transposed-u tiles as PE *stationary* and M~^T as
    moving operand, so its output lands directly in fm (no nm->fm transposes).
  * q/k/v are produced directly in nm by using G2 tiles as stationary and the
    block-diagonal Wq/Wk/Wv as moving operand.
  * Softmax normalization folded into o (scale by 1/den once per node tile).
  * x is pre-transposed to fm and cast bf16 on host; A and A^T passed bf16.
"""
import numpy as np

import concourse.bass as bass
import concourse.bacc as bacc
import concourse.mybir as mybir
from concourse.tile import TileContext
from concourse.masks import make_identity

f32 = mybir.dt.float32
bf16 = mybir.dt.bfloat16
AL = mybir.AluOpType
AF = mybir.ActivationFunctionType
AX = mybir.AxisListType

L, H, EPS = 3, 4, 1e-5
B, T, N, D, F = 16, 12, 1024, 64, 256
HD = D // H           # 16
NCORES = 8
BL = B // NCORES      # 2
NT = N // 128         # 8
TP = T // 2           # 6 t-pairs
TD = T * D            # 768
OS = 12               # out steps

DEBUG_TAPS = ()
CFG = {"offset": 2, "seq": False, "pool": False}


def _bcast(t_ap, dims, extra_off=0):
    """AP with explicit [step, count] free dims (stride-0 broadcasts allowed)."""
    return bass.AP(t_ap.tensor, t_ap.offset + extra_off,
                   [list(t_ap.ap[0])] + [list(d) for d in dims])


def build_nc(qkv_bias=False):
    nc = bacc.Bacc("TRN2", target_bir_lowering=False, debug=False)

    # x pre-transposed to fm on host: [BL, 128=(j,d), TP, N] bf16
    x_d = nc.dram_tensor("x", [BL, 128, TP, N], bf16, kind="ExternalInput")
    a_d = nc.dram_tensor("a_bf", [2, N, N], bf16, kind="ExternalInput")
    at_d = nc.dram_tensor("at_bf", [2, N, N], bf16, kind="ExternalInput")
    Wg_d = nc.dram_tensor("Wg", [L, 2 * D, D], f32, kind="ExternalInput")
    bg_d = nc.dram_tensor("bg", [L, D], f32, kind="ExternalInput")
    Wq_d = nc.dram_tensor("Wq", [L, D, D], f32, kind="ExternalInput")
    Wk_d = nc.dram_tensor("Wk", [L, D, D], f32, kind="ExternalInput")
    Wv_d = nc.dram_tensor("Wv", [L, D, D], f32, kind="ExternalInput")
    Wo_d = nc.dram_tensor("Wo", [L, D, D], f32, kind="ExternalInput")
    bo_d = nc.dram_tensor("bo", [L, D], f32, kind="ExternalInput")
    W1_d = nc.dram_tensor("W1", [L, D, F], f32, kind="ExternalInput")
    b1_d = nc.dram_tensor("b1", [L, F], f32, kind="ExternalInput")
    W2_d = nc.dram_tensor("W2", [L, F, D], f32, kind="ExternalInput")
    b2_d = nc.dram_tensor("b2", [L, D], f32, kind="ExternalInput")
    g1_d = nc.dram_tensor("ln1_g", [L, D], f32, kind="ExternalInput")
    be1_d = nc.dram_tensor("ln1_b", [L, D], f32, kind="ExternalInput")
    g2_d = nc.dram_tensor("ln2_g", [L, D], f32, kind="ExternalInput")
    be2_d = nc.dram_tensor("ln2_b", [L, D], f32, kind="ExternalInput")
    Wout_d = nc.dram_tensor("Wout", [TD, OS], f32, kind="ExternalInput")
    bout_d = nc.dram_tensor("bout", [OS], f32, kind="ExternalInput")
    bqkv_d = nc.dram_tensor("bqkv", [L, 3, D], f32, kind="ExternalInput")
    out_d = nc.dram_tensor("out", [BL, OS, N, 1], f32, kind="ExternalOutput")

    taps = {}

    def tap(name, shape, dt=bf16):
        if name is not None and name in DEBUG_TAPS:
            taps[name] = nc.dram_tensor("tap_" + name, shape, dt, kind="ExternalOutput")
            return taps[name]
        return None

    with TileContext(nc) as tc:
        with (
            tc.tile_pool(name="const", bufs=1) as cp,
            tc.tile_pool(name="wp", bufs=1) as wp,
            tc.tile_pool(name="p2", bufs=2) as p2,
            tc.tile_pool(name="p3", bufs=2) as p3,
            tc.tile_pool(name="pLN", bufs=2) as pLN,
            tc.tile_pool(name="pU", bufs=2) as pU,
            tc.tile_pool(name="pat", bufs=2) as pat,
            tc.tile_pool(name="psA", bufs=3, space="PSUM") as psA,
            tc.tile_pool(name="psQ", bufs=1, space="PSUM") as psQ,
            tc.tile_pool(name="psZ", bufs=1, space="PSUM") as psZ,
            tc.tile_pool(name="psT", bufs=1, space="PSUM") as psT,
            tc.tile_pool(name="psS", bufs=2, space="PSUM") as psS,
        ):
            # ================= persistent SBUF =================
            Bt = cp.tile([128, 2, NT, N], bf16)          # M~^T tiles (moving)
            X = cp.tile([128, BL, TP, N], bf16)          # fm state

            G2 = [cp.tile([128, TP, N], bf16, name=f"G2_{b}") for b in range(BL)]
            ONM = [cp.tile([128, NT, T, D], bf16, name=f"o_{b}") for b in range(BL)]

            ident = cp.tile([128, 128], bf16)
            make_identity(nc, ident[:])

            selS = cp.tile([128, 2], bf16)   # LN sum: sel[(j,d), j'] = 1/64 (j==j')
            nc.vector.memset(selS[:], 0.0)
            nc.vector.memset(selS[0:64, 0:1], 1.0 / 64)
            nc.vector.memset(selS[64:128, 1:2], 1.0 / 64)
            selR = cp.tile([2, 128], bf16)   # replicate: sel2[j', (j,d)] = 1 (j==j')
            pselr = psT.tile([128, 512], bf16, tag="trps", name="pselr")[:, :128]
            nc.tensor.transpose(pselr[:2], selS[:], ident[:])
            nc.scalar.mul(selR[:], pselr[:2], 64.0)

            # ---- weights ----
            Wg_bd = [[cp.tile([128, 128], bf16, name=f"Wgbd{l}_{s}") for s in range(2)]
                     for l in range(L)]
            Wq_bd = [cp.tile([128, 128], bf16, name=f"Wqbd{l}") for l in range(L)]
            Wk_bd = [cp.tile([128, 128], bf16, name=f"Wkbd{l}") for l in range(L)]
            Wv_bd = [cp.tile([128, 128], bf16, name=f"Wvbd{l}") for l in range(L)]
            Wo_bd = [cp.tile([128, 128], bf16, name=f"Wobd{l}") for l in range(L)]
            W1c = [cp.tile([128, 4, 128], bf16, name=f"W1c{l}") for l in range(L)]
            W2c = [cp.tile([128, 4, 128], bf16, name=f"W2c{l}") for l in range(L)]
            Woutc = cp.tile([128, TP, OS], bf16)
            bgv = [cp.tile([128, 1], f32, name=f"bg{l}") for l in range(L)]
            bov = [cp.tile([128, 1], f32, name=f"bo{l}") for l in range(L)]
            b1v = [cp.tile([128, 2], f32, name=f"b1{l}") for l in range(L)]
            b2v = [cp.tile([128, 1], f32, name=f"b2{l}") for l in range(L)]
            g1v = [cp.tile([128, 1], f32, name=f"g1{l}") for l in range(L)]
            be1v = [cp.tile([128, 1], f32, name=f"be1{l}") for l in range(L)]
            g2v = [cp.tile([128, 1], f32, name=f"g2{l}") for l in range(L)]
            be2v = [cp.tile([128, 1], f32, name=f"be2{l}") for l in range(L)]
            boutv = cp.tile([OS, 1], f32)
            epsv = cp.tile([128, 1], f32)
            nc.gpsimd.memset(epsv[:], EPS)
            if qkv_bias:
                bqkvr = [cp.tile([128, 3, 2, D], bf16, name=f"bqkv{l}")
                         for l in range(L)]

            def dup_bias(dst, src_ap):
                nc.gpsimd.dma_start(dst[0:64, :], src_ap[:, None])
                nc.gpsimd.dma_start(dst[64:128, :], src_ap[:, None])

            for l in range(L):
                for s in range(2):
                    nc.gpsimd.memset(Wg_bd[l][s][:], 0.0)
                    nc.gpsimd.dma_start(Wg_bd[l][s][0:64, 0:64],
                                        Wg_d.ap()[l, 64 * s:64 * (s + 1), :])
                    nc.gpsimd.dma_start(Wg_bd[l][s][64:128, 64:128],
                                        Wg_d.ap()[l, 64 * s:64 * (s + 1), :])
                for bd, wd in ((Wq_bd, Wq_d), (Wk_bd, Wk_d), (Wv_bd, Wv_d),
                               (Wo_bd, Wo_d)):
                    nc.gpsimd.memset(bd[l][:], 0.0)
                    nc.gpsimd.dma_start(bd[l][0:64, 0:64], wd.ap()[l])
                    nc.gpsimd.dma_start(bd[l][64:128, 64:128], wd.ap()[l])
                nc.gpsimd.memset(W1c[l][:], 0.0)
                nc.gpsimd.dma_start(W1c[l][0:64, 0, :], W1_d.ap()[l, :, 0:128])
                nc.gpsimd.dma_start(W1c[l][0:64, 1, :], W1_d.ap()[l, :, 128:256])
                nc.gpsimd.dma_start(W1c[l][64:128, 2, :], W1_d.ap()[l, :, 0:128])
                nc.gpsimd.dma_start(W1c[l][64:128, 3, :], W1_d.ap()[l, :, 128:256])
                nc.gpsimd.memset(W2c[l][:], 0.0)
                nc.gpsimd.dma_start(W2c[l][:, 0, 0:64], W2_d.ap()[l, 0:128, :])
                nc.gpsimd.dma_start(W2c[l][:, 1, 0:64], W2_d.ap()[l, 128:256, :])
                nc.gpsimd.dma_start(W2c[l][:, 2, 64:128], W2_d.ap()[l, 0:128, :])
                nc.gpsimd.dma_start(W2c[l][:, 3, 64:128], W2_d.ap()[l, 128:256, :])
                dup_bias(bgv[l], bg_d.ap()[l]); dup_bias(bov[l], bo_d.ap()[l])
                dup_bias(b2v[l], b2_d.ap()[l]); dup_bias(g1v[l], g1_d.ap()[l])
                dup_bias(be1v[l], be1_d.ap()[l]); dup_bias(g2v[l], g2_d.ap()[l])
                dup_bias(be2v[l], be2_d.ap()[l])
                nc.gpsimd.dma_start(b1v[l][:, 0:1], b1_d.ap()[l, 0:128][:, None])
                nc.gpsimd.dma_start(b1v[l][:, 1:2], b1_d.ap()[l, 128:256][:, None])
                if qkv_bias:
                    row = p3.tile([1, 3 * D], bf16, tag="bqrow")
                    nc.gpsimd.dma_start(
                        row[:], bqkv_d.ap()[l].rearrange("w d -> (w d)")[None, :])
                    for j in range(2):
                        nc.gpsimd.partition_broadcast(
                            bqkvr[l][:, :, j, :].rearrange("p w d -> p (w d)"),
                            row[:])
            for tpi in range(TP):
                nc.gpsimd.dma_start(
                    Woutc[:, tpi, :],
                    Wout_d.ap().rearrange("(tp p) s -> tp p s", p=128)[tpi])
            nc.gpsimd.dma_start(boutv[:], bout_d.ap()[:, None])

            # ---- x load (already fm bf16 on host)
            for bi in range(BL):
                nc.sync.dma_start(
                    X[:, bi].rearrange("p tp n -> p (tp n)"),
                    x_d.ap()[bi].rearrange("p tp n -> p (tp n)"))

            # ================= B = M~^T precompute =================
            # C := A^T (per support).  C2 = C @ C, C3 = C @ C2, computed with
            # natural-A tiles as stationary:  (C@Y)[i,n] = sum_k A[k,i] Y[k,n].
            # B'_s = 2*C3 + C2 - C   (x2 folded into the G2 relu scale).
            o0v = ONM[0][:].rearrange("p a b c -> p (a b c)").rearrange(
                "p (kb n) -> p kb n", n=N)
            o1v = ONM[1][:].rearrange("p a b c -> p (a b c)").rearrange(
                "p (kb n) -> p kb n", n=N)

            def atb(kt, sl):
                return o0v[:, kt, sl] if kt < 6 else o1v[:, kt - 6, sl]
            def c2ap(s, it, sl):
                if s == 0:
                    return Bt[:, 1, it, sl]
                return (G2[0][:, it, sl] if it < TP
                        else G2[1][:, it - TP, sl])
            for s in range(2):
                atr = at_d.ap()[s].rearrange("(kb p) n -> p kb n", p=128)
                nc.sync.dma_start(o0v, atr[:, 0:6])
                nc.sync.dma_start(o1v[:, 0:2], atr[:, 6:8])
                for pass_i in range(2):  # 0: C2 = C@C, 1: B = 2*C@C2 + C2 - C
                    for it in range(NT):
                        an = pat.tile([128, NT, 128], bf16, tag="a_natcol")
                        nc.sync.dma_start(
                            an[:], a_d.ap()[s].rearrange(
                                "(kt p) m -> p kt m", p=128)[:, :,
                                128 * it:128 * (it + 1)])
                        ps0 = psA.tile([128, 512], f32, tag="mmps", name="pb0")
                        ps1 = psA.tile([128, 512], f32, tag="mmps", name="pb1")
                        for kt in range(NT):
                            for half, ps in ((0, ps0), (1, ps1)):
                                sl = slice(512 * half, 512 * (half + 1))
                                rhs = (atb(kt, sl) if pass_i == 0
                                       else c2ap(s, kt, sl))
                                nc.tensor.matmul(ps[:], an[:, kt, :], rhs,
                                                 start=(kt == 0), stop=(kt == NT - 1))
                        for half, ps in ((0, ps0), (1, ps1)):
                            sl = slice(512 * half, 512 * (half + 1))
                            if pass_i == 0:
                                nc.scalar.copy(c2ap(s, it, sl), ps[:])
                            else:
                                tmp = p3.tile([128, 512], bf16, tag="bcomb")
                                nc.vector.tensor_tensor(
                                    tmp[:], c2ap(s, it, sl), atb(it, sl),
                                    AL.subtract)
                                nc.vector.scalar_tensor_tensor(
                                    Bt[:, s, it, sl], ps[:], 2.0, tmp[:],
                                    op0=AL.mult, op1=AL.add)
            tb = tap("Bt", [128, 2 * NT * N])
            if tb is not None:
                nc.sync.dma_start(tb.ap(), Bt[:].rearrange("p a b c -> p (a b c)"))

            # ================= stages =================
            st = {}

            def units_A(l, bi):
                """Per t-pair: u_s = X@Wg_s (fm) -> transpose tiles -> unm;
                cheb: G2 = relu(2 * B'^T-contraction + bg) directly in fm."""
                def unit(tpi):
                    unm = pU.tile([128, 2, NT, 128], bf16, tag="unm")
                    for s in range(2):
                        for ch in range(2):
                            pu = psA.tile([128, 512], f32, tag="mmps", name="pu")
                            nc.tensor.matmul(
                                pu[:], Wg_bd[l][s][:],
                                X[:, bi, tpi, 512 * ch:512 * (ch + 1)],
                                start=True, stop=True)
                            ufm = p3.tile([128, 512], bf16, tag="ufm")
                            nc.scalar.copy(ufm[:], pu[:])
                            pt = psT.tile([128, 512], bf16, tag="trps")
                            for w in range(4):
                                nc.tensor.transpose(
                                    pt[:, 128 * w:128 * (w + 1)],
                                    ufm[:, 128 * w:128 * (w + 1)], ident[:])
                            nc.scalar.copy(
                                unm[:, s, 4 * ch:4 * ch + 4, :],
                                pt[:].rearrange("p (kb n) -> p kb n", n=128))
                    ps0 = psA.tile([128, 512], f32, tag="mmps", name="pc0")
                    ps1 = psA.tile([128, 512], f32, tag="mmps", name="pc1")
                    for s in range(2):
                        for kb in range(NT):
                            first = (s == 0 and kb == 0)
                            last = (s == 1 and kb == NT - 1)
                            lhs = unm[:, s, kb, :]
                            nc.tensor.matmul(ps0[:], lhs, Bt[:, s, kb, 0:512],
                                             start=first, stop=last)
                            nc.tensor.matmul(ps1[:], lhs, Bt[:, s, kb, 512:1024],
                                             start=first, stop=last)
                    nc.scalar.activation(G2[bi][:, tpi, 0:512], ps0[:],
                                         AF.Relu, bias=bgv[l][:, 0:1], scale=2.0)
                    nc.scalar.activation(G2[bi][:, tpi, 512:1024], ps1[:],
                                         AF.Relu, bias=bgv[l][:, 0:1], scale=2.0)
                return [(lambda t=tpi: unit(t)) for tpi in range(TP)]

            def units_QC(l, bi):
                """Per node-tile: qkv direct into nm, then attention core."""
                o = ONM[bi]
                def unit(ni):
                    # ---- qkv: out[n, (j,d')] per tpi; psq free = (w, j, d')
                    qkv = p2.tile([128, 3, T, D], bf16, tag=f"qkv{bi}")
                    for tpi in range(TP):
                        psq = psQ.tile([128, 3, 2, 64], f32, tag="psq")
                        g2t = G2[bi][:, tpi, 128 * ni:128 * (ni + 1)]
                        for w, wbd in enumerate((Wq_bd, Wk_bd, Wv_bd)):
                            nc.tensor.matmul(
                                psq[:, w].rearrange("p j d -> p (j d)"),
                                g2t, wbd[l][:], start=True, stop=True)
                        dst = qkv[:, :, 2 * tpi:2 * tpi + 2, :]
                        if qkv_bias:
                            nc.vector.tensor_tensor(dst, psq[:], bqkvr[l][:], AL.add)
                        else:
                            nc.scalar.copy(dst, psq[:])
                    # ---- attention core (DVE + gpsimd)
                    qf = qkv[:, 0].rearrange("p t d -> p (t d)")
                    kf = qkv[:, 1].rearrange("p t d -> p (t d)")
                    vf = qkv[:, 2].rearrange("p t d -> p (t d)")
                    s_t = p2.tile([128, H, T, T], bf16, tag=f"s_t{bi}")
                    e_t = p2.tile([128, H, T, T], bf16, tag=f"e_t{bi}")
                    den = p2.tile([128, H, T], f32, tag=f"den{bi}")
                    rec = p2.tile([128, H, T], f32, tag=f"rec{bi}")
                    recb = p2.tile([128, H, T], bf16, tag=f"recb{bi}")

                    def qk_head(h):
                        eng = nc.gpsimd if h == 3 else nc.vector
                        prod = wp.tile([128, T, T, HD], bf16,
                                       tag=f"prodw{bi}" + ("g" if h == 3 else ""))
                        q_b = _bcast(qf, [[D, T], [0, T], [1, HD]], HD * h)
                        k_b = _bcast(kf, [[0, T], [D, T], [1, HD]], HD * h)
                        eng.tensor_tensor(prod[:], q_b, k_b, AL.mult)
                        with nc.allow_low_precision(reason="fp32 internal accum"):
                            nc.vector.tensor_reduce(
                                s_t[:, h],
                                prod[:].rearrange("p t t2 hd -> p (t t2) hd"),
                                axis=AX.X, op=AL.add)

                    def av_head(h):
                        eng = nc.gpsimd if h == 3 else nc.vector
                        prod2 = wp.tile([128, T, HD, T], bf16,
                                        tag=f"prodw{bi}" + ("g" if h == 3 else ""))
                        e_b = _bcast(e_t[:, h].rearrange("p t t2 -> p (t t2)"),
                                     [[T, T], [0, HD], [1, T]])
                        v_b = _bcast(vf, [[0, T], [1, HD], [D, T]], HD * h)
                        eng.tensor_tensor(prod2[:], e_b, v_b, AL.mult)
                        with nc.allow_low_precision(reason="fp32 internal accum"):
                            nc.vector.tensor_reduce(
                                o[:, ni, :, HD * h:HD * (h + 1)],
                                prod2[:].rearrange("p t hd t2 -> p (t hd) t2"),
                                axis=AX.X, op=AL.add)

                    def sm_wave(w):  # exp + den + rec for heads [2w, 2w+1]
                        sl = slice(2 * w, 2 * w + 2)
                        nc.scalar.activation(e_t[:, sl], s_t[:, sl], AF.Exp,
                                             scale=1.0 / (HD ** 0.5))
                        nc.vector.tensor_reduce(den[:, sl], e_t[:, sl],
                                                axis=AX.X, op=AL.add)
                        nc.vector.reciprocal_approx_fast(rec[:, sl], den[:, sl])
                        nc.vector.tensor_copy(recb[:, sl], rec[:, sl])

                    qk_head(0); qk_head(1)
                    qk_head(2); qk_head(3)
                    sm_wave(0)
                    av_head(0); av_head(1)
                    sm_wave(1)
                    av_head(2); av_head(3)
                    # o *= 1/den  (softmax normalization folded here)
                    r_b = _bcast(recb[:].rearrange("p h t -> p (h t)"),
                                 [[1, T], [T, H], [0, HD]])
                    of = o[:, ni].rearrange("p t d -> p (t d)")
                    o3 = bass.AP(of.tensor, of.offset,
                                 [list(of.ap[0]), [D, T], [HD, H], [1, HD]])
                    nc.gpsimd.tensor_tensor(o3, o3, r_b, AL.mult)
                return [(lambda n=ni: unit(n)) for ni in range(NT)]

            def _ln_fm(z_chunk, x1_dst, gv, bev, tag):
                """Post-LN in fm on a [128, 512] chunk: PE ones-matmul stats,
                centered-variance formulation (var = mean(cen^2))."""
                pm_ = psS.tile([128, 512], f32, tag="stps", name="pm_st")[:2]
                nc.tensor.matmul(pm_, selS[:], z_chunk, start=True, stop=True)
                m_sb = pLN.tile([2, 512], bf16, tag="m_sb")
                nc.scalar.copy(m_sb[:], pm_)
                pmr = psS.tile([128, 512], f32, tag="stps")
                nc.tensor.matmul(pmr[:], selR[:], m_sb[:], start=True, stop=True)
                cen = pLN.tile([128, 512], bf16, tag="cen")
                nc.vector.tensor_tensor(cen[:], z_chunk, pmr[:], AL.subtract)
                sq = pLN.tile([128, 512], bf16, tag="sq")
                nc.scalar.square(sq[:], cen[:])
                pv = psS.tile([128, 512], f32, tag="stps", name="pv_st")[:2]
                nc.tensor.matmul(pv, selS[:], sq[:], start=True, stop=True)
                sd = pLN.tile([2, 512], f32, tag="sd")
                nc.scalar.activation(sd[:], pv, AF.Sqrt, bias=epsv[:2, 0:1])
                rstdf = pLN.tile([2, 512], f32, tag="rstdf")
                nc.vector.reciprocal_approx_fast(rstdf[:], sd[:])
                rstd = pLN.tile([2, 512], bf16, tag="rstd")
                nc.vector.tensor_copy(rstd[:], rstdf[:])
                prr = psS.tile([128, 512], f32, tag="stps")
                nc.tensor.matmul(prr[:], selR[:], rstd[:], start=True, stop=True)
                xh = sq
                nc.vector.tensor_tensor(xh[:], cen[:], prr[:], AL.mult)  # overwrites sq
                nc.scalar.activation(x1_dst, xh[:], AF.Identity,
                                     bias=bev[:, 0:1], scale=gv[:, 0:1])

            def units_D(l, bi):
                """o->fm; Wo+res; LN1; FFN+res; LN2 -> X (all fm)."""
                o, g2 = ONM[bi], G2[bi]
                def unit(tpi, ch):
                    if True:
                        pt = psT.tile([128, 512], bf16, tag="trps")
                        for w in range(4):
                            ni = 4 * ch + w
                            nc.tensor.transpose(
                                pt[:, 128 * w:128 * (w + 1)],
                                o[:, ni, 2 * tpi:2 * tpi + 2, :]
                                .rearrange("p t d -> p (t d)"),
                                ident[:])
                        ofm = wp.tile([128, 512], bf16, tag="ofm")
                        nc.scalar.copy(ofm[:], pt[:])
                        po = psA.tile([128, 512], f32, tag="mmps", name="po")
                        nc.tensor.matmul(po[:], Wo_bd[l][:], ofm[:],
                                         start=True, stop=True)
                        g2s = g2[:, tpi, 512 * ch:512 * (ch + 1)]
                        # x1 = G2 + (wo_out + bo)   (in-place)
                        nc.vector.scalar_tensor_tensor(
                            g2s, po[:], bov[l][:, 0:1], g2s, op0=AL.add, op1=AL.add)
                        x1t = p3.tile([128, 512], bf16, tag="x1n")
                        x1ns = x1t[:]
                        _ln_fm(g2s, x1ns, g1v[l], be1v[l], "1")
                        # FFN
                        pz = psZ.tile([128, 512], f32, tag="zps")
                        for c in range(4):
                            pmid = psA.tile([128, 512], f32, tag="mmps", name="pmid")
                            nc.tensor.matmul(pmid[:], W1c[l][:, c], x1ns,
                                             start=True, stop=True)
                            mid = p3.tile([128, 512], bf16, tag="mid")
                            nc.scalar.activation(mid[:], pmid[:], AF.Relu,
                                                 bias=b1v[l][:, c % 2:c % 2 + 1])
                            nc.tensor.matmul(pz[:], W2c[l][:, c], mid[:],
                                             start=(c == 0), stop=(c == 3))
                        # z = x1n + (w2_out + b2)  (stored into G2 slot)
                        nc.vector.scalar_tensor_tensor(
                            g2s, pz[:], b2v[l][:, 0:1], x1ns, op0=AL.add, op1=AL.add)
                        # LN2 -> X (fm)
                        _ln_fm(g2s, X[:, bi, tpi, 512 * ch:512 * (ch + 1)],
                               g2v[l], be2v[l], "2")
                return [(lambda t=tpi, c=ch: unit(t, c))
                        for tpi in range(TP) for ch in range(2)]

            def stage_F(bi):
                outsb = wp.tile([OS, N], bf16, tag="outsb")
                for ch in range(2):
                    pf = psA.tile([128, 512], f32, tag="mmps", name="pf_out")[:OS]
                    for tpi in range(TP):
                        nc.tensor.matmul(
                            pf, Woutc[:, tpi],
                            X[:, bi, tpi, 512 * ch:512 * (ch + 1)],
                            start=(tpi == 0), stop=(tpi == TP - 1))
                    nc.scalar.activation(outsb[:, 512 * ch:512 * (ch + 1)], pf,
                                         AF.Identity, bias=boutv[:, 0:1])
                nc.gpsimd.dma_start(out_d.ap()[bi].rearrange("s n o -> s (n o)"),
                                    outsb[:])

            # -------- emission: 2-stream stage interleave --------
            def stage_A(l, bi):
                for u in units_A(l, bi): u()

            def stage_QC(l, bi):
                for u in units_QC(l, bi): u()

            def stage_D(l, bi):
                for u in units_D(l, bi): u()

            prog = {b: [] for b in range(BL)}
            for l in range(L):
                for b in range(BL):
                    prog[b] += [(stage_A, l, b), (stage_QC, l, b), (stage_D, l, b)]
            for b in range(BL):
                prog[b].append((stage_F, b))
            order = []
            i0 = i1 = 0
            OFFSET = CFG["offset"]
            while i0 < len(prog[0]) or i1 < len(prog[1]):
                if i0 < len(prog[0]) and (i0 - OFFSET < i1 or i1 >= len(prog[1])):
                    order.append(prog[0][i0]); i0 += 1
                else:
                    order.append(prog[1][i1]); i1 += 1
            for fn, *args in order:
                fn(*args)

    nc.compile()
    return nc, taps


_CACHE = {}


def _get_nc(qkv_bias=False):
    key = ("nc", qkv_bias)
    if key not in _CACHE:
        _CACHE[key] = build_nc(qkv_bias)
    return _CACHE[key]


def _prep_inputs(inputs):
    import ml_dtypes
    bf = ml_dtypes.bfloat16
    x = np.asarray(inputs["x"], dtype=np.float32)         # [B, T, N, D]
    # fm layout: [B, (j=t%2, d), tp, n]
    x_fm = np.ascontiguousarray(
        x.reshape(B, TP, 2, N, D).transpose(0, 2, 4, 1, 3)
        .reshape(B, 128, TP, N)).astype(bf)
    sup = np.asarray(inputs["supports"], dtype=np.float32)
    a_bf = np.ascontiguousarray(sup).astype(bf)
    at_bf = np.ascontiguousarray(sup.transpose(0, 2, 1)).astype(bf)
    bqkv = np.stack([np.asarray(inputs["bq"], np.float32),
                     np.asarray(inputs["bk"], np.float32),
                     np.asarray(inputs["bv"], np.float32)], axis=1)  # [L,3,D]
    shared = {"a_bf": a_bf, "at_bf": at_bf,
              "bqkv": np.ascontiguousarray(bqkv)}
    names = ["Wg", "bg", "Wq", "Wk", "Wv", "Wo", "bo", "W1", "b1", "W2", "b2",
             "ln1_g", "ln1_b", "ln2_g", "ln2_b", "Wout", "bout"]
    for n in names:
        shared[n] = np.ascontiguousarray(np.asarray(inputs[n], dtype=np.float32))
    qkv_bias = bool(np.any(bqkv))
    in_maps = []
    for c in range(NCORES):
        m = dict(shared)
        m["x"] = np.ascontiguousarray(x_fm[c * BL:(c + 1) * BL])
        in_maps.append(m)
    return in_maps, qkv_bias


def kernel(**inputs):
    from concourse.bass_utils import run_bass_kernel_spmd
    in_maps, qkv_bias = _prep_inputs(inputs)
    nc, taps = _get_nc(qkv_bias)
    res = run_bass_kernel_spmd(nc, in_maps, core_ids=list(range(NCORES)))
    _CACHE["last_res"] = res
    out = np.concatenate([r["out"] for r in res.results], axis=0)
    return out.astype(np.float32)



# revision 18
# speedup vs baseline: 1.2337x; 1.2337x over previous
"""TRN2 Bass kernel for nn_ST_model_58815282151899 (dense ST-transformer).

Sharding: data-parallel over batch (B=16 -> 2 per core x 8 cores, no collectives).

Key structure (vs naive):
  * Chebyshev collapse: sum_{k<4} T_k(A) = 4A^3 + 2A^2 - 2A =: M (per support).
    M~ = [M1 M2] is precomputed ONCE on device and kept SBUF-resident (bf16).
    Per layer the GNN is then   G = relu(M1 (x Wg1) + M2 (x Wg2) + bg)
    = one 2048-deep PSUM-accumulated matmul (feature transform applied first).
  * Layouts: feature-major fm = [(j=t%2, d) part, tp=t//2, n] for all linears
    and both layernorms (PE ones-matmul stats); node-major nm = [n%128 part,
    (t, d) free] only for the attention core (DVE broadcast ops).
  * The cheb matmul uses transposed-u tiles as PE *stationary* and M~^T as
    moving operand, so its output lands directly in fm (no nm->fm transposes).
  * q/k/v are produced directly in nm by using G2 tiles as stationary and the
    block-diagonal Wq/Wk/Wv as moving operand.
  * Softmax normalization folded into o (scale by 1/den once per node tile).
  * x is pre-transposed to fm and cast bf16 on host; A and A^T passed bf16.
"""
import numpy as np

import concourse.bass as bass
import concourse.bacc as bacc
import concourse.mybir as mybir
from concourse.tile import TileContext
from concourse.masks import make_identity

f32 = mybir.dt.float32
bf16 = mybir.dt.bfloat16
AL = mybir.AluOpType
AF = mybir.ActivationFunctionType
AX = mybir.AxisListType

L, H, EPS = 3, 4, 1e-5
B, T, N, D, F = 16, 12, 1024, 64, 256
HD = D // H           # 16
NCORES = 8
BL = B // NCORES      # 2
NT = N // 128         # 8
TP = T // 2           # 6 t-pairs
TD = T * D            # 768
OS = 12               # out steps

DEBUG_TAPS = ()
CFG = {"offset": 2, "seq": False, "pool": False}


def _bcast(t_ap, dims, extra_off=0):
    """AP with explicit [step, count] free dims (stride-0 broadcasts allowed)."""
    return bass.AP(t_ap.tensor, t_ap.offset + extra_off,
                   [list(t_ap.ap[0])] + [list(d) for d in dims])


def build_nc(qkv_bias=False):
    nc = bacc.Bacc("TRN2", target_bir_lowering=False, debug=False)

    # x pre-transposed to fm on host: [BL, 128=(j,d), TP, N] bf16
    x_d = nc.dram_tensor("x", [BL, 128, TP, N], bf16, kind="ExternalInput")
    a_d = nc.dram_tensor("a_bf", [2, N, N], bf16, kind="ExternalInput")
    at_d = nc.dram_tensor("at_bf", [2, N, N], bf16, kind="ExternalInput")
    Wg_d = nc.dram_tensor("Wg", [L, 2 * D, D], f32, kind="ExternalInput")
    bg_d = nc.dram_tensor("bg", [L, D], f32, kind="ExternalInput")
    Wq_d = nc.dram_tensor("Wq", [L, D, D], f32, kind="ExternalInput")
    Wk_d = nc.dram_tensor("Wk", [L, D, D], f32, kind="ExternalInput")
    Wv_d = nc.dram_tensor("Wv", [L, D, D], f32, kind="ExternalInput")
    Wo_d = nc.dram_tensor("Wo", [L, D, D], f32, kind="ExternalInput")
    bo_d = nc.dram_tensor("bo", [L, D], f32, kind="ExternalInput")
    W1_d = nc.dram_tensor("W1", [L, D, F], f32, kind="ExternalInput")
    b1_d = nc.dram_tensor("b1", [L, F], f32, kind="ExternalInput")
    W2_d = nc.dram_tensor("W2", [L, F, D], f32, kind="ExternalInput")
    b2_d = nc.dram_tensor("b2", [L, D], f32, kind="ExternalInput")
    g1_d = nc.dram_tensor("ln1_g", [L, D], f32, kind="ExternalInput")
    be1_d = nc.dram_tensor("ln1_b", [L, D], f32, kind="ExternalInput")
    g2_d = nc.dram_tensor("ln2_g", [L, D], f32, kind="ExternalInput")
    be2_d = nc.dram_tensor("ln2_b", [L, D], f32, kind="ExternalInput")
    Wout_d = nc.dram_tensor("Wout", [TD, OS], f32, kind="ExternalInput")
    bout_d = nc.dram_tensor("bout", [OS], f32, kind="ExternalInput")
    bqkv_d = nc.dram_tensor("bqkv", [L, 3, D], f32, kind="ExternalInput")
    out_d = nc.dram_tensor("out", [BL, OS, N, 1], f32, kind="ExternalOutput")

    taps = {}

    def tap(name, shape, dt=bf16):
        if name is not None and name in DEBUG_TAPS:
            taps[name] = nc.dram_tensor("tap_" + name, shape, dt, kind="ExternalOutput")
            return taps[name]
        return None

    with TileContext(nc) as tc:
        with (
            tc.tile_pool(name="const", bufs=1) as cp,
            tc.tile_pool(name="wp", bufs=1) as wp,
            tc.tile_pool(name="p2", bufs=2) as p2,
            tc.tile_pool(name="p3", bufs=2) as p3,
            tc.tile_pool(name="pLN", bufs=2) as pLN,
            tc.tile_pool(name="pU", bufs=2) as pU,
            tc.tile_pool(name="pat", bufs=2) as pat,
            tc.tile_pool(name="psA", bufs=3, space="PSUM") as psA,
            tc.tile_pool(name="psQ", bufs=1, space="PSUM") as psQ,
            tc.tile_pool(name="psZ", bufs=1, space="PSUM") as psZ,
            tc.tile_pool(name="psT", bufs=1, space="PSUM") as psT,
            tc.tile_pool(name="psS", bufs=2, space="PSUM") as psS,
        ):
            # ================= persistent SBUF =================
            Bt = cp.tile([128, 2, NT, N], bf16)          # M~^T tiles (moving)
            X = cp.tile([128, BL, TP, N], bf16)          # fm state

            G2 = [cp.tile([128, TP, N], bf16, name=f"G2_{b}") for b in range(BL)]
            ONM = [cp.tile([128, NT, T, D], bf16, name=f"o_{b}") for b in range(BL)]

            ident = cp.tile([128, 128], bf16)
            make_identity(nc, ident[:])

            selS = cp.tile([128, 2], bf16)   # LN sum: sel[(j,d), j'] = 1/64 (j==j')
            nc.vector.memset(selS[:], 0.0)
            nc.vector.memset(selS[0:64, 0:1], 1.0 / 64)
            nc.vector.memset(selS[64:128, 1:2], 1.0 / 64)
            selR = cp.tile([2, 128], bf16)   # replicate: sel2[j', (j,d)] = 1 (j==j')
            pselr = psT.tile([128, 512], bf16, tag="trps", name="pselr")[:, :128]
            nc.tensor.transpose(pselr[:2], selS[:], ident[:])
            nc.scalar.mul(selR[:], pselr[:2], 64.0)

            # ---- weights ----
            Wg_bd = [[cp.tile([128, 128], bf16, name=f"Wgbd{l}_{s}") for s in range(2)]
                     for l in range(L)]
            Wqkv_bd = [cp.tile([128, 3, 128], bf16, name=f"Wqkvbd{l}")
                       for l in range(L)]
            Wo_bd = [cp.tile([128, 128], bf16, name=f"Wobd{l}") for l in range(L)]
            W1c = [cp.tile([128, 4, 128], bf16, name=f"W1c{l}") for l in range(L)]
            W2c = [cp.tile([128, 4, 128], bf16, name=f"W2c{l}") for l in range(L)]
            Woutc = cp.tile([128, TP, OS], bf16)
            bgv = [cp.tile([128, 1], f32, name=f"bg{l}") for l in range(L)]
            bov = [cp.tile([128, 1], f32, name=f"bo{l}") for l in range(L)]
            b1v = [cp.tile([128, 2], f32, name=f"b1{l}") for l in range(L)]
            b2v = [cp.tile([128, 1], f32, name=f"b2{l}") for l in range(L)]
            g1v = [cp.tile([128, 1], f32, name=f"g1{l}") for l in range(L)]
            be1v = [cp.tile([128, 1], f32, name=f"be1{l}") for l in range(L)]
            g2v = [cp.tile([128, 1], f32, name=f"g2{l}") for l in range(L)]
            be2v = [cp.tile([128, 1], f32, name=f"be2{l}") for l in range(L)]
            boutv = cp.tile([OS, 1], f32)
            epsv = cp.tile([128, 1], f32)
            nc.gpsimd.memset(epsv[:], EPS)
            if qkv_bias:
                bqkvr = [cp.tile([128, 3, 2, D], bf16, name=f"bqkv{l}")
                         for l in range(L)]

            def dup_bias(dst, src_ap):
                nc.gpsimd.dma_start(dst[0:64, :], src_ap[:, None])
                nc.gpsimd.dma_start(dst[64:128, :], src_ap[:, None])

            for l in range(L):
                for s in range(2):
                    nc.gpsimd.memset(Wg_bd[l][s][:], 0.0)
                    nc.gpsimd.dma_start(Wg_bd[l][s][0:64, 0:64],
                                        Wg_d.ap()[l, 64 * s:64 * (s + 1), :])
                    nc.gpsimd.dma_start(Wg_bd[l][s][64:128, 64:128],
                                        Wg_d.ap()[l, 64 * s:64 * (s + 1), :])
                nc.gpsimd.memset(Wqkv_bd[l][:], 0.0)
                for w, wd in enumerate((Wq_d, Wk_d, Wv_d)):
                    nc.gpsimd.dma_start(Wqkv_bd[l][0:64, w, 0:64], wd.ap()[l])
                    nc.gpsimd.dma_start(Wqkv_bd[l][64:128, w, 64:128], wd.ap()[l])
                nc.gpsimd.memset(Wo_bd[l][:], 0.0)
                nc.gpsimd.dma_start(Wo_bd[l][0:64, 0:64], Wo_d.ap()[l])
                nc.gpsimd.dma_start(Wo_bd[l][64:128, 64:128], Wo_d.ap()[l])
                nc.gpsimd.memset(W1c[l][:], 0.0)
                nc.gpsimd.dma_start(W1c[l][0:64, 0, :], W1_d.ap()[l, :, 0:128])
                nc.gpsimd.dma_start(W1c[l][0:64, 1, :], W1_d.ap()[l, :, 128:256])
                nc.gpsimd.dma_start(W1c[l][64:128, 2, :], W1_d.ap()[l, :, 0:128])
                nc.gpsimd.dma_start(W1c[l][64:128, 3, :], W1_d.ap()[l, :, 128:256])
                nc.gpsimd.memset(W2c[l][:], 0.0)
                nc.gpsimd.dma_start(W2c[l][:, 0, 0:64], W2_d.ap()[l, 0:128, :])
                nc.gpsimd.dma_start(W2c[l][:, 1, 0:64], W2_d.ap()[l, 128:256, :])
                nc.gpsimd.dma_start(W2c[l][:, 2, 64:128], W2_d.ap()[l, 0:128, :])
                nc.gpsimd.dma_start(W2c[l][:, 3, 64:128], W2_d.ap()[l, 128:256, :])
                dup_bias(bgv[l], bg_d.ap()[l]); dup_bias(bov[l], bo_d.ap()[l])
                dup_bias(b2v[l], b2_d.ap()[l]); dup_bias(g1v[l], g1_d.ap()[l])
                dup_bias(be1v[l], be1_d.ap()[l]); dup_bias(g2v[l], g2_d.ap()[l])
                dup_bias(be2v[l], be2_d.ap()[l])
                nc.gpsimd.dma_start(b1v[l][:, 0:1], b1_d.ap()[l, 0:128][:, None])
                nc.gpsimd.dma_start(b1v[l][:, 1:2], b1_d.ap()[l, 128:256][:, None])
                if qkv_bias:
                    row = p3.tile([1, 3 * D], bf16, tag="bqrow")
                    nc.gpsimd.dma_start(
                        row[:], bqkv_d.ap()[l].rearrange("w d -> (w d)")[None, :])
                    for j in range(2):
                        nc.gpsimd.partition_broadcast(
                            bqkvr[l][:, :, j, :].rearrange("p w d -> p (w d)"),
                            row[:])
            for tpi in range(TP):
                nc.gpsimd.dma_start(
                    Woutc[:, tpi, :],
                    Wout_d.ap().rearrange("(tp p) s -> tp p s", p=128)[tpi])
            nc.gpsimd.dma_start(boutv[:], bout_d.ap()[:, None])

            # ---- x load (already fm bf16 on host)
            for bi in range(BL):
                nc.sync.dma_start(
                    X[:, bi].rearrange("p tp n -> p (tp n)"),
                    x_d.ap()[bi].rearrange("p tp n -> p (tp n)"))

            # ================= B = M~^T precompute =================
            # C := A^T (per support).  C2 = C @ C, C3 = C @ C2, computed with
            # natural-A tiles as stationary:  (C@Y)[i,n] = sum_k A[k,i] Y[k,n].
            # B'_s = 2*C3 + C2 - C   (x2 folded into the G2 relu scale).
            o0v = ONM[0][:].rearrange("p a b c -> p (a b c)").rearrange(
                "p (kb n) -> p kb n", n=N)
            o1v = ONM[1][:].rearrange("p a b c -> p (a b c)").rearrange(
                "p (kb n) -> p kb n", n=N)

            def atb(kt, sl):
                return o0v[:, kt, sl] if kt < 6 else o1v[:, kt - 6, sl]
            def c2ap(s, it, sl):
                if s == 0:
                    return Bt[:, 1, it, sl]
                return (G2[0][:, it, sl] if it < TP
                        else G2[1][:, it - TP, sl])
            for s in range(2):
                atr = at_d.ap()[s].rearrange("(kb p) n -> p kb n", p=128)
                nc.sync.dma_start(o0v, atr[:, 0:6])
                nc.sync.dma_start(o1v[:, 0:2], atr[:, 6:8])
                for pass_i in range(2):  # 0: C2 = C@C, 1: B = 2*C@C2 + C2 - C
                    for it in range(NT):
                        an = pat.tile([128, NT, 128], bf16, tag="a_natcol")
                        nc.sync.dma_start(
                            an[:], a_d.ap()[s].rearrange(
                                "(kt p) m -> p kt m", p=128)[:, :,
                                128 * it:128 * (it + 1)])
                        ps0 = psA.tile([128, 512], f32, tag="mmps", name="pb0")
                        ps1 = psA.tile([128, 512], f32, tag="mmps", name="pb1")
                        for kt in range(NT):
                            for half, ps in ((0, ps0), (1, ps1)):
                                sl = slice(512 * half, 512 * (half + 1))
                                rhs = (atb(kt, sl) if pass_i == 0
                                       else c2ap(s, kt, sl))
                                nc.tensor.matmul(ps[:], an[:, kt, :], rhs,
                                                 start=(kt == 0), stop=(kt == NT - 1))
                        for half, ps in ((0, ps0), (1, ps1)):
                            sl = slice(512 * half, 512 * (half + 1))
                            if pass_i == 0:
                                nc.scalar.copy(c2ap(s, it, sl), ps[:])
                            else:
                                tmp = p3.tile([128, 512], bf16, tag="bcomb")
                                nc.vector.tensor_tensor(
                                    tmp[:], c2ap(s, it, sl), atb(it, sl),
                                    AL.subtract)
                                nc.vector.scalar_tensor_tensor(
                                    Bt[:, s, it, sl], ps[:], 2.0, tmp[:],
                                    op0=AL.mult, op1=AL.add)
            tb = tap("Bt", [128, 2 * NT * N])
            if tb is not None:
                nc.sync.dma_start(tb.ap(), Bt[:].rearrange("p a b c -> p (a b c)"))

            # ================= stages =================
            st = {}

            def units_A(l, bi):
                """Per t-pair: u_s = X@Wg_s (fm) -> transpose tiles -> unm;
                cheb: G2 = relu(2 * B'^T-contraction + bg) directly in fm."""
                def unit(tpi):
                    unm = pU.tile([128, 2, NT, 128], bf16, tag="unm")
                    for s in range(2):
                        for ch in range(2):
                            pu = psA.tile([128, 512], f32, tag="mmps", name="pu")
                            nc.tensor.matmul(
                                pu[:], Wg_bd[l][s][:],
                                X[:, bi, tpi, 512 * ch:512 * (ch + 1)],
                                start=True, stop=True)
                            ufm = p3.tile([128, 512], bf16, tag="ufm")
                            nc.scalar.copy(ufm[:], pu[:])
                            pt = psT.tile([128, 512], bf16, tag="trps")
                            for w in range(4):
                                nc.tensor.transpose(
                                    pt[:, 128 * w:128 * (w + 1)],
                                    ufm[:, 128 * w:128 * (w + 1)], ident[:])
                            nc.scalar.copy(
                                unm[:, s, 4 * ch:4 * ch + 4, :],
                                pt[:].rearrange("p (kb n) -> p kb n", n=128))
                    ps0 = psA.tile([128, 512], f32, tag="mmps", name="pc0")
                    ps1 = psA.tile([128, 512], f32, tag="mmps", name="pc1")
                    for s in range(2):
                        for kb in range(NT):
                            first = (s == 0 and kb == 0)
                            last = (s == 1 and kb == NT - 1)
                            lhs = unm[:, s, kb, :]
                            nc.tensor.matmul(ps0[:], lhs, Bt[:, s, kb, 0:512],
                                             start=first, stop=last)
                            nc.tensor.matmul(ps1[:], lhs, Bt[:, s, kb, 512:1024],
                                             start=first, stop=last)
                    nc.scalar.activation(G2[bi][:, tpi, 0:512], ps0[:],
                                         AF.Relu, bias=bgv[l][:, 0:1], scale=2.0)
                    nc.scalar.activation(G2[bi][:, tpi, 512:1024], ps1[:],
                                         AF.Relu, bias=bgv[l][:, 0:1], scale=2.0)
                return [(lambda t=tpi: unit(t)) for tpi in range(TP)]

            def units_QC(l, bi):
                """Per node-tile: qkv into head-major nm layouts, then the
                attention core as 2x-mode DVE ops with halving-tree reduces."""
                o = ONM[bi]
                def unit(ni):
                    # ---- qkv -> QK [w,h,t,hd] head-major; VT [h,hd,t2] ----
                    QK = p2.tile([128, 2, H, T, HD], bf16, tag=f"qk{bi}")
                    VT = p2.tile([128, H, HD, T], bf16, tag=f"vt{bi}")
                    QKf = QK[:].rearrange("p a b c d -> p (a b c d)")
                    VTf = VT[:].rearrange("p a b c -> p (a b c)")
                    for tpi in range(TP):
                        psq = psQ.tile([128, 3, 2, 64], f32, tag="psq")
                        g2t = G2[bi][:, tpi, 128 * ni:128 * (ni + 1)]
                        nc.tensor.matmul(
                            psq[:].rearrange("p w j d -> p (w j d)"),
                            g2t, Wqkv_bd[l][:].rearrange("p w d -> p (w d)"),
                            start=True, stop=True)
                        psqf = psq[:].rearrange("p w j d -> p (w j d)")
                        # q,k: iterate (j, h, hd); t = 2*tpi + j
                        for w in range(2):
                            nc.scalar.copy(
                                _bcast(QKf, [[HD, 2], [T * HD, H], [1, HD]],
                                       w * T * D + 2 * tpi * HD),
                                _bcast(psqf, [[64, 2], [HD, H], [1, HD]],
                                       w * 128))
                        # v: iterate (j, h, hd); t2 = 2*tpi + j
                        nc.scalar.copy(
                            _bcast(VTf, [[1, 2], [HD * T, H], [T, HD]], 2 * tpi),
                            _bcast(psqf, [[64, 2], [HD, H], [1, HD]], 256))
                    if qkv_bias:
                        nc.gpsimd.tensor_tensor(
                            _bcast(QKf, [[T * D, 2], [T * HD, H], [HD, T],
                                         [1, HD]]),
                            _bcast(QKf, [[T * D, 2], [T * HD, H], [HD, T],
                                         [1, HD]]),
                            _bcast(bqkvr[l][:].rearrange("p w j d -> p (w j d)"),
                                   [[2 * D, 2], [HD, H], [0, T], [1, HD]]),
                            AL.add)
                        nc.gpsimd.tensor_tensor(
                            _bcast(VTf, [[HD * T, H], [T, HD], [1, T]]),
                            _bcast(VTf, [[HD * T, H], [T, HD], [1, T]]),
                            _bcast(bqkvr[l][:].rearrange("p w j d -> p (w j d)"),
                                   [[HD, H], [1, HD], [0, T]], 2 * 2 * D),
                            AL.add)
                    # ---- attention core ----
                    s_t = p2.tile([128, H, T, T], bf16, tag=f"s_t{bi}")
                    e_t = p2.tile([128, H, T, T], bf16, tag=f"e_t{bi}")
                    den = p2.tile([128, H, T], f32, tag=f"den{bi}")
                    rec = p2.tile([128, H, T], f32, tag=f"rec{bi}")
                    recb = p2.tile([128, H, T], bf16, tag=f"recb{bi}")
                    Sf = s_t[:].rearrange("p a b c -> p (a b c)")
                    Ef = e_t[:].rearrange("p a b c -> p (a b c)")

                    def qk_head(h):
                        off = h * T * HD
                        prod = wp.tile([128, T, T, HD], bf16,
                                       tag=f"prodw{bi}")
                        pf = prod[:].rearrange("p a b c -> p (a b c)")
                        nc.vector.tensor_tensor(
                            pf,
                            _bcast(QKf, [[HD, T], [0, T], [1, HD]], off),
                            _bcast(QKf, [[0, T], [1, T * HD]], T * D + off),
                            AL.mult)
                        h1 = wp.tile([128, 1152], bf16, tag=f"h1w{bi}")
                        nc.vector.tensor_tensor(
                            h1[:], _bcast(pf, [[16, 144], [1, 8]], 0),
                            _bcast(pf, [[16, 144], [1, 8]], 8), AL.add)
                        h2 = pf[:, 0:576]
                        h1f = h1[:]
                        nc.vector.tensor_tensor(
                            h2, _bcast(h1f, [[8, 144], [1, 4]], 0),
                            _bcast(h1f, [[8, 144], [1, 4]], 4), AL.add)
                        with nc.allow_low_precision(reason="qk tree-sum"):
                            nc.vector.tensor_reduce(
                                s_t[:, h],
                                h2.rearrange("p (a b) -> p a b", b=4),
                                axis=AX.X, op=AL.add)

                    def sm_pair(hp):
                        sl = slice(2 * hp, 2 * hp + 2)
                        nc.scalar.activation(e_t[:, sl], s_t[:, sl], AF.Exp,
                                             scale=1.0 / (HD ** 0.5))
                        nc.vector.tensor_reduce(den[:, sl], e_t[:, sl],
                                                axis=AX.X, op=AL.add)
                        nc.vector.reciprocal_approx_fast(rec[:, sl], den[:, sl])
                        nc.vector.tensor_copy(recb[:, sl], rec[:, sl])

                    def av_head(h):
                        prod = wp.tile([128, T, HD, T], bf16,
                                       tag=f"prodw{bi}")
                        pf = prod[:].rearrange("p a b c -> p (a b c)")
                        nc.vector.tensor_tensor(
                            pf,
                            _bcast(Ef, [[T, T], [0, HD], [1, T]], h * T * T),
                            _bcast(VTf, [[0, T], [1, HD * T]], h * HD * T),
                            AL.mult)
                        h1 = wp.tile([128, 1152], bf16, tag=f"h1w{bi}")
                        nc.vector.tensor_tensor(
                            h1[:], _bcast(pf, [[12, 192], [1, 6]], 0),
                            _bcast(pf, [[12, 192], [1, 6]], 6), AL.add)
                        h2 = pf[:, 0:576]
                        h1f = h1[:]
                        nc.vector.tensor_tensor(
                            h2, _bcast(h1f, [[6, 192], [1, 3]], 0),
                            _bcast(h1f, [[6, 192], [1, 3]], 3), AL.add)
                        of = o[:].rearrange("p a b c -> p (a b c)")
                        with nc.allow_low_precision(reason="av tree-sum"):
                            nc.vector.tensor_reduce(
                                _bcast(of, [[D, T], [1, HD]],
                                       ni * T * D + h * HD),
                                h2.rearrange("p (a b) -> p a b", b=3),
                                axis=AX.X, op=AL.add)

                    qk_head(0); qk_head(1)
                    sm_pair(0)
                    qk_head(2); qk_head(3)
                    av_head(0); av_head(1)
                    sm_pair(1)
                    av_head(2); av_head(3)
                    # o *= 1/den  (softmax normalization folded here)
                    r_b = _bcast(recb[:].rearrange("p h t -> p (h t)"),
                                 [[1, T], [T, H], [0, HD]])
                    of = o[:, ni].rearrange("p t d -> p (t d)")
                    o3 = bass.AP(of.tensor, of.offset,
                                 [list(of.ap[0]), [D, T], [HD, H], [1, HD]])
                    nc.gpsimd.tensor_tensor(o3, o3, r_b, AL.mult)
                return [(lambda n=ni: unit(n)) for ni in range(NT)]

            def _ln_fm(z_chunk, x1_dst, gv, bev, tag):
                """Post-LN in fm on a [128, 512] chunk: PE ones-matmul stats,
                centered-variance formulation (var = mean(cen^2))."""
                pm_ = psS.tile([128, 512], f32, tag="stps", name="pm_st")[:2]
                nc.tensor.matmul(pm_, selS[:], z_chunk, start=True, stop=True)
                m_sb = pLN.tile([2, 512], bf16, tag="m_sb")
                nc.scalar.copy(m_sb[:], pm_)
                pmr = psS.tile([128, 512], f32, tag="stps")
                nc.tensor.matmul(pmr[:], selR[:], m_sb[:], start=True, stop=True)
                cen = pLN.tile([128, 512], bf16, tag="cen")
                nc.vector.tensor_tensor(cen[:], z_chunk, pmr[:], AL.subtract)
                sq = pLN.tile([128, 512], bf16, tag="sq")
                nc.scalar.square(sq[:], cen[:])
                pv = psS.tile([128, 512], f32, tag="stps", name="pv_st")[:2]
                nc.tensor.matmul(pv, selS[:], sq[:], start=True, stop=True)
                sd = pLN.tile([2, 512], f32, tag="sd")
                nc.scalar.activation(sd[:], pv, AF.Sqrt, bias=epsv[:2, 0:1])
                rstdf = pLN.tile([2, 512], f32, tag="rstdf")
                nc.vector.reciprocal_approx_fast(rstdf[:], sd[:])
                rstd = pLN.tile([2, 512], bf16, tag="rstd")
                nc.gpsimd.tensor_copy(rstd[:], rstdf[:])
                prr = psS.tile([128, 512], f32, tag="stps")
                nc.tensor.matmul(prr[:], selR[:], rstd[:], start=True, stop=True)
                xh = sq
                nc.vector.tensor_tensor(xh[:], cen[:], prr[:], AL.mult)  # overwrites sq
                nc.scalar.activation(x1_dst, xh[:], AF.Identity,
                                     bias=bev[:, 0:1], scale=gv[:, 0:1])

            def units_D(l, bi):
                """o->fm; Wo+res; LN1; FFN+res; LN2 -> X (all fm)."""
                o, g2 = ONM[bi], G2[bi]
                def unit(tpi, ch):
                    if True:
                        pt = psT.tile([128, 512], bf16, tag="trps")
                        for w in range(4):
                            ni = 4 * ch + w
                            nc.tensor.transpose(
                                pt[:, 128 * w:128 * (w + 1)],
                                o[:, ni, 2 * tpi:2 * tpi + 2, :]
                                .rearrange("p t d -> p (t d)"),
                                ident[:])
                        ofm = wp.tile([128, 512], bf16, tag="ofm")
                        nc.scalar.copy(ofm[:], pt[:])
                        po = psA.tile([128, 512], f32, tag="mmps", name="po")
                        nc.tensor.matmul(po[:], Wo_bd[l][:], ofm[:],
                                         start=True, stop=True)
                        g2s = g2[:, tpi, 512 * ch:512 * (ch + 1)]
                        # x1 = G2 + (wo_out + bo)   (in-place)
                        nc.vector.scalar_tensor_tensor(
                            g2s, po[:], bov[l][:, 0:1], g2s, op0=AL.add, op1=AL.add)
                        x1t = p3.tile([128, 512], bf16, tag="x1n")
                        x1ns = x1t[:]
                        _ln_fm(g2s, x1ns, g1v[l], be1v[l], "1")
                        # FFN
                        pz = psZ.tile([128, 512], f32, tag="zps")
                        for c in range(4):
                            pmid = psA.tile([128, 512], f32, tag="mmps", name="pmid")
                            nc.tensor.matmul(pmid[:], W1c[l][:, c], x1ns,
                                             start=True, stop=True)
                            mid = p3.tile([128, 512], bf16, tag="mid")
                            nc.scalar.activation(mid[:], pmid[:], AF.Relu,
                                                 bias=b1v[l][:, c % 2:c % 2 + 1])
                            nc.tensor.matmul(pz[:], W2c[l][:, c], mid[:],
                                             start=(c == 0), stop=(c == 3))
                        # z = x1n + (w2_out + b2)  (stored into G2 slot)
                        nc.vector.scalar_tensor_tensor(
                            g2s, pz[:], b2v[l][:, 0:1], x1ns, op0=AL.add, op1=AL.add)
                        # LN2 -> X (fm)
                        _ln_fm(g2s, X[:, bi, tpi, 512 * ch:512 * (ch + 1)],
                               g2v[l], be2v[l], "2")
                return [(lambda t=tpi, c=ch: unit(t, c))
                        for tpi in range(TP) for ch in range(2)]

            def stage_F(bi):
                outsb = wp.tile([OS, N], bf16, tag="outsb")
                for ch in range(2):
                    pf = psA.tile([128, 512], f32, tag="mmps", name="pf_out")[:OS]
                    for tpi in range(TP):
                        nc.tensor.matmul(
                            pf, Woutc[:, tpi],
                            X[:, bi, tpi, 512 * ch:512 * (ch + 1)],
                            start=(tpi == 0), stop=(tpi == TP - 1))
                    nc.scalar.activation(outsb[:, 512 * ch:512 * (ch + 1)], pf,
                                         AF.Identity, bias=boutv[:, 0:1])
                nc.gpsimd.dma_start(out_d.ap()[bi].rearrange("s n o -> s (n o)"),
                                    outsb[:])

            # -------- emission: 2-stream stage interleave --------
            def stage_A(l, bi):
                for u in units_A(l, bi): u()

            def stage_QC(l, bi):
                for u in units_QC(l, bi): u()

            def stage_D(l, bi):
                for u in units_D(l, bi): u()

            prog = {b: [] for b in range(BL)}
            for l in range(L):
                for b in range(BL):
                    prog[b] += [(stage_A, l, b), (stage_QC, l, b), (stage_D, l, b)]
            for b in range(BL):
                prog[b].append((stage_F, b))
            order = []
            i0 = i1 = 0
            OFFSET = CFG["offset"]
            while i0 < len(prog[0]) or i1 < len(prog[1]):
                if i0 < len(prog[0]) and (i0 - OFFSET < i1 or i1 >= len(prog[1])):
                    order.append(prog[0][i0]); i0 += 1
                else:
                    order.append(prog[1][i1]); i1 += 1
            for fn, *args in order:
                fn(*args)

    nc.compile()
    return nc, taps


_CACHE = {}


def _get_nc(qkv_bias=False):
    key = ("nc", qkv_bias)
    if key not in _CACHE:
        _CACHE[key] = build_nc(qkv_bias)
    return _CACHE[key]


def _prep_inputs(inputs):
    import ml_dtypes
    bf = ml_dtypes.bfloat16
    x = np.asarray(inputs["x"], dtype=np.float32)         # [B, T, N, D]
    # fm layout: [B, (j=t%2, d), tp, n]
    x_fm = np.ascontiguousarray(
        x.reshape(B, TP, 2, N, D).transpose(0, 2, 4, 1, 3)
        .reshape(B, 128, TP, N)).astype(bf)
    sup = np.asarray(inputs["supports"], dtype=np.float32)
    a_bf = np.ascontiguousarray(sup).astype(bf)
    at_bf = np.ascontiguousarray(sup.transpose(0, 2, 1)).astype(bf)
    bqkv = np.stack([np.asarray(inputs["bq"], np.float32),
                     np.asarray(inputs["bk"], np.float32),
                     np.asarray(inputs["bv"], np.float32)], axis=1)  # [L,3,D]
    shared = {"a_bf": a_bf, "at_bf": at_bf,
              "bqkv": np.ascontiguousarray(bqkv)}
    names = ["Wg", "bg", "Wq", "Wk", "Wv", "Wo", "bo", "W1", "b1", "W2", "b2",
             "ln1_g", "ln1_b", "ln2_g", "ln2_b", "Wout", "bout"]
    for n in names:
        shared[n] = np.ascontiguousarray(np.asarray(inputs[n], dtype=np.float32))
    qkv_bias = bool(np.any(bqkv))
    in_maps = []
    for c in range(NCORES):
        m = dict(shared)
        m["x"] = np.ascontiguousarray(x_fm[c * BL:(c + 1) * BL])
        in_maps.append(m)
    return in_maps, qkv_bias


def kernel(**inputs):
    from concourse.bass_utils import run_bass_kernel_spmd
    in_maps, qkv_bias = _prep_inputs(inputs)
    nc, taps = _get_nc(qkv_bias)
    res = run_bass_kernel_spmd(nc, in_maps, core_ids=list(range(NCORES)))
    _CACHE["last_res"] = res
    out = np.concatenate([r["out"] for r in res.results], axis=0)
    return out.astype(np.float32)



# revision 20
# speedup vs baseline: 1.2980x; 1.0521x over previous
"""TRN2 Bass kernel for nn_ST_model_58815282151899 (dense ST-transformer).

Sharding: data-parallel over batch (B=16 -> 2 per core x 8 cores, no collectives).

Key structure (vs naive):
  * Chebyshev collapse: sum_{k<4} T_k(A) = 4A^3 + 2A^2 - 2A =: M (per support).
    M~ = [M1 M2] is precomputed ONCE on device and kept SBUF-resident (bf16).
    Per layer the GNN is then   G = relu(M1 (x Wg1) + M2 (x Wg2) + bg)
    = one 2048-deep PSUM-accumulated matmul (feature transform applied first).
  * Layouts: feature-major fm = [(j=t%2, d) part, tp=t//2, n] for all linears
    and both layernorms (PE ones-matmul stats); node-major nm = [n%128 part,
    (t, d) free] only for the attention core (DVE broadcast ops).
  * The cheb matmul uses transposed-u tiles as PE *stationary* and M~^T as
    moving operand, so its output lands directly in fm (no nm->fm transposes).
  * q/k/v are produced directly in nm by using G2 tiles as stationary and the
    block-diagonal Wq/Wk/Wv as moving operand.
  * Softmax normalization folded into o (scale by 1/den once per node tile).
  * x is pre-transposed to fm and cast bf16 on host; A and A^T passed bf16.
"""
import numpy as np

import concourse.bass as bass
import concourse.bacc as bacc
import concourse.mybir as mybir
from concourse.tile import TileContext
from concourse.masks import make_identity

f32 = mybir.dt.float32
bf16 = mybir.dt.bfloat16
AL = mybir.AluOpType
AF = mybir.ActivationFunctionType
AX = mybir.AxisListType

L, H, EPS = 3, 4, 1e-5
B, T, N, D, F = 16, 12, 1024, 64, 256
HD = D // H           # 16
NCORES = 8
BL = B // NCORES      # 2
NT = N // 128         # 8
TP = T // 2           # 6 t-pairs
TD = T * D            # 768
OS = 12               # out steps

DEBUG_TAPS = ()
CFG = {"offset": 2, "seq": False, "pool": False}


def _bcast(t_ap, dims, extra_off=0):
    """AP with explicit [step, count] free dims (stride-0 broadcasts allowed)."""
    return bass.AP(t_ap.tensor, t_ap.offset + extra_off,
                   [list(t_ap.ap[0])] + [list(d) for d in dims])


def build_nc(qkv_bias=False):
    nc = bacc.Bacc("TRN2", target_bir_lowering=False, debug=False)

    # x pre-transposed to fm on host: [BL, 128=(j,d), TP, N] bf16
    x_d = nc.dram_tensor("x", [BL, 128, TP, N], bf16, kind="ExternalInput")
    a_d = nc.dram_tensor("a_bf", [2, N, N], bf16, kind="ExternalInput")
    at_d = nc.dram_tensor("at_bf", [2, N, N], bf16, kind="ExternalInput")
    Wg_d = nc.dram_tensor("Wg", [L, 2 * D, D], f32, kind="ExternalInput")
    bg_d = nc.dram_tensor("bg", [L, D], f32, kind="ExternalInput")
    Wq_d = nc.dram_tensor("Wq", [L, D, D], f32, kind="ExternalInput")
    Wk_d = nc.dram_tensor("Wk", [L, D, D], f32, kind="ExternalInput")
    Wv_d = nc.dram_tensor("Wv", [L, D, D], f32, kind="ExternalInput")
    Wo_d = nc.dram_tensor("Wo", [L, D, D], f32, kind="ExternalInput")
    bo_d = nc.dram_tensor("bo", [L, D], f32, kind="ExternalInput")
    W1_d = nc.dram_tensor("W1", [L, D, F], f32, kind="ExternalInput")
    b1_d = nc.dram_tensor("b1", [L, F], f32, kind="ExternalInput")
    W2_d = nc.dram_tensor("W2", [L, F, D], f32, kind="ExternalInput")
    b2_d = nc.dram_tensor("b2", [L, D], f32, kind="ExternalInput")
    g1_d = nc.dram_tensor("ln1_g", [L, D], f32, kind="ExternalInput")
    be1_d = nc.dram_tensor("ln1_b", [L, D], f32, kind="ExternalInput")
    g2_d = nc.dram_tensor("ln2_g", [L, D], f32, kind="ExternalInput")
    be2_d = nc.dram_tensor("ln2_b", [L, D], f32, kind="ExternalInput")
    Wout_d = nc.dram_tensor("Wout", [TD, OS], f32, kind="ExternalInput")
    bout_d = nc.dram_tensor("bout", [OS], f32, kind="ExternalInput")
    bqkv_d = nc.dram_tensor("bqkv", [L, 3, D], f32, kind="ExternalInput")
    out_d = nc.dram_tensor("out", [BL, OS, N, 1], f32, kind="ExternalOutput")

    taps = {}

    def tap(name, shape, dt=bf16):
        if name is not None and name in DEBUG_TAPS:
            taps[name] = nc.dram_tensor("tap_" + name, shape, dt, kind="ExternalOutput")
            return taps[name]
        return None

    with TileContext(nc) as tc:
        with (
            tc.tile_pool(name="const", bufs=1) as cp,
            tc.tile_pool(name="wp", bufs=1) as wp,
            tc.tile_pool(name="p2", bufs=2) as p2,
            tc.tile_pool(name="p3", bufs=2) as p3,
            tc.tile_pool(name="pLN", bufs=2) as pLN,
            tc.tile_pool(name="pU", bufs=2) as pU,
            tc.tile_pool(name="pat", bufs=2) as pat,
            tc.tile_pool(name="psA", bufs=2, space="PSUM") as psA,
            tc.tile_pool(name="psQ", bufs=1, space="PSUM") as psQ,
            tc.tile_pool(name="psZ", bufs=1, space="PSUM") as psZ,
            tc.tile_pool(name="psT", bufs=1, space="PSUM") as psT,
            tc.tile_pool(name="psS", bufs=3, space="PSUM") as psS,
        ):
            # ================= persistent SBUF =================
            Bt = cp.tile([128, 2, NT, N], bf16)          # M~^T tiles (moving)
            X = cp.tile([128, BL, TP, N], bf16)          # fm state

            G2 = [cp.tile([128, TP, N], bf16, name=f"G2_{b}") for b in range(BL)]
            ONM = [cp.tile([128, NT, T, D], bf16, name=f"o_{b}") for b in range(BL)]

            ident = cp.tile([128, 128], bf16)
            make_identity(nc, ident[:])

            selS = cp.tile([128, 2], bf16)   # LN sum: sel[(j,d), j'] = 1/64 (j==j')
            nc.vector.memset(selS[:], 0.0)
            nc.vector.memset(selS[0:64, 0:1], 1.0 / 64)
            nc.vector.memset(selS[64:128, 1:2], 1.0 / 64)
            selR = cp.tile([2, 128], bf16)   # replicate: sel2[j', (j,d)] = 1 (j==j')
            pselr = psT.tile([128, 512], bf16, tag="trps", name="pselr")[:, :128]
            nc.tensor.transpose(pselr[:2], selS[:], ident[:])
            nc.scalar.mul(selR[:], pselr[:2], 64.0)

            # ---- weights ----
            Wg_bd = [[cp.tile([128, 128], bf16, name=f"Wgbd{l}_{s}") for s in range(2)]
                     for l in range(L)]
            Wqkv_bd = [cp.tile([128, 3, 128], bf16, name=f"Wqkvbd{l}")
                       for l in range(L)]
            Wo_bd = [cp.tile([128, 128], bf16, name=f"Wobd{l}") for l in range(L)]
            W1c = [cp.tile([128, 4, 128], bf16, name=f"W1c{l}") for l in range(L)]
            W2c = [cp.tile([128, 4, 128], bf16, name=f"W2c{l}") for l in range(L)]
            Woutc = cp.tile([128, TP, OS], bf16)
            bgv = [cp.tile([128, 1], f32, name=f"bg{l}") for l in range(L)]
            bov = [cp.tile([128, 1], f32, name=f"bo{l}") for l in range(L)]
            b1v = [cp.tile([128, 2], f32, name=f"b1{l}") for l in range(L)]
            b2v = [cp.tile([128, 1], f32, name=f"b2{l}") for l in range(L)]
            g1v = [cp.tile([128, 1], f32, name=f"g1{l}") for l in range(L)]
            be1v = [cp.tile([128, 1], f32, name=f"be1{l}") for l in range(L)]
            g2v = [cp.tile([128, 1], f32, name=f"g2{l}") for l in range(L)]
            be2v = [cp.tile([128, 1], f32, name=f"be2{l}") for l in range(L)]
            boutv = cp.tile([OS, 1], f32)
            epsv = cp.tile([128, 1], f32)
            nc.gpsimd.memset(epsv[:], EPS)
            if qkv_bias:
                bqkvr = [cp.tile([128, 3, 2, D], bf16, name=f"bqkv{l}")
                         for l in range(L)]

            def dup_bias(dst, src_ap):
                nc.gpsimd.dma_start(dst[0:64, :], src_ap[:, None])
                nc.gpsimd.dma_start(dst[64:128, :], src_ap[:, None])

            for l in range(L):
                for s in range(2):
                    nc.gpsimd.memset(Wg_bd[l][s][:], 0.0)
                    nc.gpsimd.dma_start(Wg_bd[l][s][0:64, 0:64],
                                        Wg_d.ap()[l, 64 * s:64 * (s + 1), :])
                    nc.gpsimd.dma_start(Wg_bd[l][s][64:128, 64:128],
                                        Wg_d.ap()[l, 64 * s:64 * (s + 1), :])
                nc.gpsimd.memset(Wqkv_bd[l][:], 0.0)
                for w, wd in enumerate((Wq_d, Wk_d, Wv_d)):
                    nc.gpsimd.dma_start(Wqkv_bd[l][0:64, w, 0:64], wd.ap()[l])
                    nc.gpsimd.dma_start(Wqkv_bd[l][64:128, w, 64:128], wd.ap()[l])
                nc.gpsimd.memset(Wo_bd[l][:], 0.0)
                nc.gpsimd.dma_start(Wo_bd[l][0:64, 0:64], Wo_d.ap()[l])
                nc.gpsimd.dma_start(Wo_bd[l][64:128, 64:128], Wo_d.ap()[l])
                nc.gpsimd.memset(W1c[l][:], 0.0)
                nc.gpsimd.dma_start(W1c[l][0:64, 0, :], W1_d.ap()[l, :, 0:128])
                nc.gpsimd.dma_start(W1c[l][0:64, 1, :], W1_d.ap()[l, :, 128:256])
                nc.gpsimd.dma_start(W1c[l][64:128, 2, :], W1_d.ap()[l, :, 0:128])
                nc.gpsimd.dma_start(W1c[l][64:128, 3, :], W1_d.ap()[l, :, 128:256])
                nc.gpsimd.memset(W2c[l][:], 0.0)
                nc.gpsimd.dma_start(W2c[l][:, 0, 0:64], W2_d.ap()[l, 0:128, :])
                nc.gpsimd.dma_start(W2c[l][:, 1, 0:64], W2_d.ap()[l, 128:256, :])
                nc.gpsimd.dma_start(W2c[l][:, 2, 64:128], W2_d.ap()[l, 0:128, :])
                nc.gpsimd.dma_start(W2c[l][:, 3, 64:128], W2_d.ap()[l, 128:256, :])
                dup_bias(bgv[l], bg_d.ap()[l]); dup_bias(bov[l], bo_d.ap()[l])
                dup_bias(b2v[l], b2_d.ap()[l]); dup_bias(g1v[l], g1_d.ap()[l])
                dup_bias(be1v[l], be1_d.ap()[l]); dup_bias(g2v[l], g2_d.ap()[l])
                dup_bias(be2v[l], be2_d.ap()[l])
                nc.gpsimd.dma_start(b1v[l][:, 0:1], b1_d.ap()[l, 0:128][:, None])
                nc.gpsimd.dma_start(b1v[l][:, 1:2], b1_d.ap()[l, 128:256][:, None])
                if qkv_bias:
                    row = p3.tile([1, 3 * D], bf16, tag="bqrow")
                    nc.gpsimd.dma_start(
                        row[:], bqkv_d.ap()[l].rearrange("w d -> (w d)")[None, :])
                    for j in range(2):
                        nc.gpsimd.partition_broadcast(
                            bqkvr[l][:, :, j, :].rearrange("p w d -> p (w d)"),
                            row[:])
            for tpi in range(TP):
                nc.gpsimd.dma_start(
                    Woutc[:, tpi, :],
                    Wout_d.ap().rearrange("(tp p) s -> tp p s", p=128)[tpi])
            nc.gpsimd.dma_start(boutv[:], bout_d.ap()[:, None])

            # ---- x load (already fm bf16 on host)
            for bi in range(BL):
                nc.sync.dma_start(
                    X[:, bi].rearrange("p tp n -> p (tp n)"),
                    x_d.ap()[bi].rearrange("p tp n -> p (tp n)"))

            # ================= B = M~^T precompute =================
            # C := A^T (per support).  C2 = C @ C, C3 = C @ C2, computed with
            # natural-A tiles as stationary:  (C@Y)[i,n] = sum_k A[k,i] Y[k,n].
            # B'_s = 2*C3 + C2 - C   (x2 folded into the G2 relu scale).
            o0v = ONM[0][:].rearrange("p a b c -> p (a b c)").rearrange(
                "p (kb n) -> p kb n", n=N)
            o1v = ONM[1][:].rearrange("p a b c -> p (a b c)").rearrange(
                "p (kb n) -> p kb n", n=N)

            def atb(kt, sl):
                return o0v[:, kt, sl] if kt < 6 else o1v[:, kt - 6, sl]
            def c2ap(s, it, sl):
                if s == 0:
                    return Bt[:, 1, it, sl]
                return (G2[0][:, it, sl] if it < TP
                        else G2[1][:, it - TP, sl])
            for s in range(2):
                atr = at_d.ap()[s].rearrange("(kb p) n -> p kb n", p=128)
                nc.sync.dma_start(o0v, atr[:, 0:6])
                nc.sync.dma_start(o1v[:, 0:2], atr[:, 6:8])
                for pass_i in range(2):  # 0: C2 = C@C, 1: B = 2*C@C2 + C2 - C
                    for it in range(NT):
                        an = pat.tile([128, NT, 128], bf16, tag="a_natcol")
                        nc.sync.dma_start(
                            an[:], a_d.ap()[s].rearrange(
                                "(kt p) m -> p kt m", p=128)[:, :,
                                128 * it:128 * (it + 1)])
                        ps0 = psA.tile([128, 512], f32, tag="mmps", name="pb0")
                        ps1 = psA.tile([128, 512], f32, tag="mmps", name="pb1")
                        for kt in range(NT):
                            for half, ps in ((0, ps0), (1, ps1)):
                                sl = slice(512 * half, 512 * (half + 1))
                                rhs = (atb(kt, sl) if pass_i == 0
                                       else c2ap(s, kt, sl))
                                nc.tensor.matmul(ps[:], an[:, kt, :], rhs,
                                                 start=(kt == 0), stop=(kt == NT - 1))
                        for half, ps in ((0, ps0), (1, ps1)):
                            sl = slice(512 * half, 512 * (half + 1))
                            if pass_i == 0:
                                nc.scalar.copy(c2ap(s, it, sl), ps[:])
                            else:
                                tmp = p3.tile([128, 512], bf16, tag="bcomb")
                                nc.vector.tensor_tensor(
                                    tmp[:], c2ap(s, it, sl), atb(it, sl),
                                    AL.subtract)
                                nc.vector.scalar_tensor_tensor(
                                    Bt[:, s, it, sl], ps[:], 2.0, tmp[:],
                                    op0=AL.mult, op1=AL.add)
            tb = tap("Bt", [128, 2 * NT * N])
            if tb is not None:
                nc.sync.dma_start(tb.ap(), Bt[:].rearrange("p a b c -> p (a b c)"))

            # ================= stages =================
            st = {}

            def units_A(l, bi):
                """Per t-pair: u_s = X@Wg_s (fm) -> transpose tiles -> unm;
                cheb: G2 = relu(2 * B'^T-contraction + bg) directly in fm."""
                def unit(tpi):
                    unm = pU.tile([128, 2, NT, 128], bf16, tag="unm")
                    # u directly in nm: stationary = X node-slice, moving = Wg
                    unmf = unm[:].rearrange("p s kb d -> p (s kb d)")
                    for np_ in range(4):  # ni-pairs
                        puq = psA.tile([128, 4, 128], f32, tag="mmps",
                                       name="puq")
                        for i in range(2):
                            ni = 2 * np_ + i
                            xs = X[:, bi, tpi, 128 * ni:128 * (ni + 1)]
                            for s in range(2):
                                nc.tensor.matmul(puq[:, 2 * i + s, :], xs,
                                                 Wg_bd[l][s][:],
                                                 start=True, stop=True)
                        # bank layout (ni-sub, s, d') -> unm (s, kb, d')
                        nc.scalar.copy(
                            _bcast(unmf, [[128, 2], [NT * 128, 2], [1, 128]],
                                   2 * np_ * 128),
                            puq[:].rearrange("p a b -> p (a b)"))
                    ps0 = psA.tile([128, 512], f32, tag="mmps", name="pc0")
                    ps1 = psA.tile([128, 512], f32, tag="mmps", name="pc1")
                    for s in range(2):
                        for kb in range(NT):
                            first = (s == 0 and kb == 0)
                            last = (s == 1 and kb == NT - 1)
                            lhs = unm[:, s, kb, :]
                            nc.tensor.matmul(ps0[:], lhs, Bt[:, s, kb, 0:512],
                                             start=first, stop=last)
                            nc.tensor.matmul(ps1[:], lhs, Bt[:, s, kb, 512:1024],
                                             start=first, stop=last)
                    nc.scalar.activation(G2[bi][:, tpi, 0:512], ps0[:],
                                         AF.Relu, bias=bgv[l][:, 0:1], scale=2.0)
                    nc.scalar.activation(G2[bi][:, tpi, 512:1024], ps1[:],
                                         AF.Relu, bias=bgv[l][:, 0:1], scale=2.0)
                return [(lambda t=tpi: unit(t)) for tpi in range(TP)]

            def units_QC(l, bi):
                """Per node-tile: qkv into head-major nm layouts, then the
                attention core as 2x-mode DVE ops with halving-tree reduces."""
                o = ONM[bi]
                def unit(ni):
                    # ---- qkv -> QK [w,h,t,hd] head-major; VT [h,hd,t2] ----
                    QK = p2.tile([128, 2, H, T, HD], bf16, tag=f"qk{bi}")
                    VT = p2.tile([128, H, HD, T], bf16, tag=f"vt{bi}")
                    QKf = QK[:].rearrange("p a b c d -> p (a b c d)")
                    VTf = VT[:].rearrange("p a b c -> p (a b c)")
                    for tpi in range(TP):
                        psq = psQ.tile([128, 3, 2, 64], f32, tag="psq")
                        g2t = G2[bi][:, tpi, 128 * ni:128 * (ni + 1)]
                        nc.tensor.matmul(
                            psq[:].rearrange("p w j d -> p (w j d)"),
                            g2t, Wqkv_bd[l][:].rearrange("p w d -> p (w d)"),
                            start=True, stop=True)
                        psqf = psq[:].rearrange("p w j d -> p (w j d)")
                        # q,k: iterate (j, h, hd); t = 2*tpi + j
                        for w in range(2):
                            nc.scalar.copy(
                                _bcast(QKf, [[HD, 2], [T * HD, H], [1, HD]],
                                       w * T * D + 2 * tpi * HD),
                                _bcast(psqf, [[64, 2], [HD, H], [1, HD]],
                                       w * 128))
                        # v: iterate (j, h, hd); t2 = 2*tpi + j
                        nc.scalar.copy(
                            _bcast(VTf, [[1, 2], [HD * T, H], [T, HD]], 2 * tpi),
                            _bcast(psqf, [[64, 2], [HD, H], [1, HD]], 256))
                    if qkv_bias:
                        nc.gpsimd.tensor_tensor(
                            _bcast(QKf, [[T * D, 2], [T * HD, H], [HD, T],
                                         [1, HD]]),
                            _bcast(QKf, [[T * D, 2], [T * HD, H], [HD, T],
                                         [1, HD]]),
                            _bcast(bqkvr[l][:].rearrange("p w j d -> p (w j d)"),
                                   [[2 * D, 2], [HD, H], [0, T], [1, HD]]),
                            AL.add)
                        nc.gpsimd.tensor_tensor(
                            _bcast(VTf, [[HD * T, H], [T, HD], [1, T]]),
                            _bcast(VTf, [[HD * T, H], [T, HD], [1, T]]),
                            _bcast(bqkvr[l][:].rearrange("p w j d -> p (w j d)"),
                                   [[HD, H], [1, HD], [0, T]], 2 * 2 * D),
                            AL.add)
                    # ---- attention core ----
                    s_t = p2.tile([128, H, T, T], bf16, tag=f"s_t{bi}")
                    e_t = p2.tile([128, H, T, T], bf16, tag=f"e_t{bi}")
                    den = p2.tile([128, H, T], f32, tag=f"den{bi}")
                    rec = p2.tile([128, H, T], f32, tag=f"rec{bi}")
                    recb = p2.tile([128, H, T], bf16, tag=f"recb{bi}")
                    Sf = s_t[:].rearrange("p a b c -> p (a b c)")
                    Ef = e_t[:].rearrange("p a b c -> p (a b c)")

                    def qk_head(h):
                        off = h * T * HD
                        prod = wp.tile([128, T, T, HD], bf16,
                                       tag=f"prodw{bi}")
                        pf = prod[:].rearrange("p a b c -> p (a b c)")
                        nc.vector.tensor_tensor(
                            pf,
                            _bcast(QKf, [[HD, T], [0, T], [1, HD]], off),
                            _bcast(QKf, [[0, T], [1, T * HD]], T * D + off),
                            AL.mult)
                        h1 = wp.tile([128, 1152], bf16, tag=f"h1w{bi}")
                        nc.vector.tensor_tensor(
                            h1[:], _bcast(pf, [[16, 144], [1, 8]], 0),
                            _bcast(pf, [[16, 144], [1, 8]], 8), AL.add)
                        h2 = pf[:, 0:576]
                        h1f = h1[:]
                        nc.vector.tensor_tensor(
                            h2, _bcast(h1f, [[8, 144], [1, 4]], 0),
                            _bcast(h1f, [[8, 144], [1, 4]], 4), AL.add)
                        with nc.allow_low_precision(reason="qk tree-sum"):
                            nc.vector.tensor_reduce(
                                s_t[:, h],
                                h2.rearrange("p (a b) -> p a b", b=4),
                                axis=AX.X, op=AL.add)

                    def sm_pair(hp):
                        sl = slice(2 * hp, 2 * hp + 2)
                        nc.scalar.activation(e_t[:, sl], s_t[:, sl], AF.Exp,
                                             scale=1.0 / (HD ** 0.5))
                        nc.vector.tensor_reduce(den[:, sl], e_t[:, sl],
                                                axis=AX.X, op=AL.add)
                        nc.vector.reciprocal_approx_fast(rec[:, sl], den[:, sl])
                        nc.vector.tensor_copy(recb[:, sl], rec[:, sl])

                    def av_head(h):
                        prod = wp.tile([128, T, HD, T], bf16,
                                       tag=f"prodw{bi}")
                        pf = prod[:].rearrange("p a b c -> p (a b c)")
                        nc.vector.tensor_tensor(
                            pf,
                            _bcast(Ef, [[T, T], [0, HD], [1, T]], h * T * T),
                            _bcast(VTf, [[0, T], [1, HD * T]], h * HD * T),
                            AL.mult)
                        h1 = wp.tile([128, 1152], bf16, tag=f"h1w{bi}")
                        nc.vector.tensor_tensor(
                            h1[:], _bcast(pf, [[12, 192], [1, 6]], 0),
                            _bcast(pf, [[12, 192], [1, 6]], 6), AL.add)
                        h2 = pf[:, 0:576]
                        h1f = h1[:]
                        nc.vector.tensor_tensor(
                            h2, _bcast(h1f, [[6, 192], [1, 3]], 0),
                            _bcast(h1f, [[6, 192], [1, 3]], 3), AL.add)
                        of = o[:].rearrange("p a b c -> p (a b c)")
                        with nc.allow_low_precision(reason="av tree-sum"):
                            nc.vector.tensor_reduce(
                                _bcast(of, [[D, T], [1, HD]],
                                       ni * T * D + h * HD),
                                h2.rearrange("p (a b) -> p a b", b=3),
                                axis=AX.X, op=AL.add)

                    qk_head(0); qk_head(1)
                    sm_pair(0)
                    qk_head(2); qk_head(3)
                    av_head(0); av_head(1)
                    sm_pair(1)
                    av_head(2); av_head(3)
                    # o *= 1/den  (softmax normalization folded here)
                    r_b = _bcast(recb[:].rearrange("p h t -> p (h t)"),
                                 [[1, T], [T, H], [0, HD]])
                    of = o[:, ni].rearrange("p t d -> p (t d)")
                    o3 = bass.AP(of.tensor, of.offset,
                                 [list(of.ap[0]), [D, T], [HD, H], [1, HD]])
                    nc.gpsimd.tensor_tensor(o3, o3, r_b, AL.mult)
                return [(lambda n=ni: unit(n)) for ni in range(NT)]

            def _ln_fm(z_chunk, x1_dst, gv, bev, tag):
                """Post-LN in fm on a [128, 512] chunk: PE ones-matmul stats,
                centered-variance formulation (var = mean(cen^2))."""
                pm_ = psS.tile([128, 512], f32, tag="stps", name="pm_st")[:2]
                nc.tensor.matmul(pm_, selS[:], z_chunk, start=True, stop=True)
                m_sb = pLN.tile([2, 512], bf16, tag="m_sb")
                nc.scalar.copy(m_sb[:], pm_)
                pmr = psS.tile([128, 512], f32, tag="stps")
                nc.tensor.matmul(pmr[:], selR[:], m_sb[:], start=True, stop=True)
                cen = pLN.tile([128, 512], bf16, tag="cen")
                nc.vector.tensor_tensor(cen[:], z_chunk, pmr[:], AL.subtract)
                sq = pLN.tile([128, 512], bf16, tag="sq")
                nc.scalar.square(sq[:], cen[:])
                pv = psS.tile([128, 512], f32, tag="stps", name="pv_st")[:2]
                nc.tensor.matmul(pv, selS[:], sq[:], start=True, stop=True)
                sd = pLN.tile([2, 512], f32, tag="sd")
                nc.scalar.activation(sd[:], pv, AF.Sqrt, bias=epsv[:2, 0:1])
                rstdf = pLN.tile([2, 512], f32, tag="rstdf")
                nc.vector.reciprocal_approx_fast(rstdf[:], sd[:])
                rstd = pLN.tile([2, 512], bf16, tag="rstd")
                nc.vector.tensor_copy(rstd[:], rstdf[:])
                prr = psS.tile([128, 512], f32, tag="stps")
                nc.tensor.matmul(prr[:], selR[:], rstd[:], start=True, stop=True)
                xh = sq
                nc.vector.tensor_tensor(xh[:], cen[:], prr[:], AL.mult)  # overwrites sq
                nc.scalar.activation(x1_dst, xh[:], AF.Identity,
                                     bias=bev[:, 0:1], scale=gv[:, 0:1])

            def units_D(l, bi):
                """o->fm; Wo+res; LN1; FFN+res; LN2 -> X (all fm)."""
                o, g2 = ONM[bi], G2[bi]
                def unit(tpi, ch):
                    if True:
                        pt = psT.tile([128, 512], bf16, tag="trps")
                        for w in range(4):
                            ni = 4 * ch + w
                            nc.tensor.transpose(
                                pt[:, 128 * w:128 * (w + 1)],
                                o[:, ni, 2 * tpi:2 * tpi + 2, :]
                                .rearrange("p t d -> p (t d)"),
                                ident[:])
                        ofm = wp.tile([128, 512], bf16, tag="ofm")
                        nc.scalar.copy(ofm[:], pt[:])
                        po = psA.tile([128, 512], f32, tag="mmps", name="po")
                        nc.tensor.matmul(po[:], Wo_bd[l][:], ofm[:],
                                         start=True, stop=True)
                        g2s = g2[:, tpi, 512 * ch:512 * (ch + 1)]
                        # x1 = G2 + (wo_out + bo)   (in-place)
                        nc.vector.scalar_tensor_tensor(
                            g2s, po[:], bov[l][:, 0:1], g2s, op0=AL.add, op1=AL.add)
                        x1t = p3.tile([128, 512], bf16, tag="x1n")
                        x1ns = x1t[:]
                        _ln_fm(g2s, x1ns, g1v[l], be1v[l], "1")
                        # FFN
                        pz = psZ.tile([128, 512], f32, tag="zps")
                        for c in range(4):
                            pmid = psA.tile([128, 512], f32, tag="mmps", name="pmid")
                            nc.tensor.matmul(pmid[:], W1c[l][:, c], x1ns,
                                             start=True, stop=True)
                            mid = p3.tile([128, 512], bf16, tag="mid")
                            nc.scalar.activation(mid[:], pmid[:], AF.Relu,
                                                 bias=b1v[l][:, c % 2:c % 2 + 1])
                            nc.tensor.matmul(pz[:], W2c[l][:, c], mid[:],
                                             start=(c == 0), stop=(c == 3))
                        # z = x1n + (w2_out + b2)  (stored into G2 slot)
                        nc.vector.scalar_tensor_tensor(
                            g2s, pz[:], b2v[l][:, 0:1], x1ns, op0=AL.add, op1=AL.add)
                        # LN2 -> X (fm)
                        _ln_fm(g2s, X[:, bi, tpi, 512 * ch:512 * (ch + 1)],
                               g2v[l], be2v[l], "2")
                return [(lambda t=tpi, c=ch: unit(t, c))
                        for tpi in range(TP) for ch in range(2)]

            def stage_F(bi):
                outsb = wp.tile([OS, N], bf16, tag="outsb")
                for ch in range(2):
                    pf = psA.tile([128, 512], f32, tag="mmps", name="pf_out")[:OS]
                    for tpi in range(TP):
                        nc.tensor.matmul(
                            pf, Woutc[:, tpi],
                            X[:, bi, tpi, 512 * ch:512 * (ch + 1)],
                            start=(tpi == 0), stop=(tpi == TP - 1))
                    nc.scalar.activation(outsb[:, 512 * ch:512 * (ch + 1)], pf,
                                         AF.Identity, bias=boutv[:, 0:1])
                nc.gpsimd.dma_start(out_d.ap()[bi].rearrange("s n o -> s (n o)"),
                                    outsb[:])

            # -------- emission: 2-stream stage interleave --------
            def stage_A(l, bi):
                for u in units_A(l, bi): u()

            def stage_QC(l, bi):
                for u in units_QC(l, bi): u()

            def stage_D(l, bi):
                for u in units_D(l, bi): u()

            prog = {b: [] for b in range(BL)}
            for l in range(L):
                for b in range(BL):
                    prog[b] += [(stage_A, l, b), (stage_QC, l, b), (stage_D, l, b)]
            for b in range(BL):
                prog[b].append((stage_F, b))
            order = []
            i0 = i1 = 0
            OFFSET = CFG["offset"]
            while i0 < len(prog[0]) or i1 < len(prog[1]):
                if i0 < len(prog[0]) and (i0 - OFFSET < i1 or i1 >= len(prog[1])):
                    order.append(prog[0][i0]); i0 += 1
                else:
                    order.append(prog[1][i1]); i1 += 1
            for fn, *args in order:
                fn(*args)

    nc.compile()
    return nc, taps


_CACHE = {}


def _get_nc(qkv_bias=False):
    key = ("nc", qkv_bias)
    if key not in _CACHE:
        _CACHE[key] = build_nc(qkv_bias)
    return _CACHE[key]


def _prep_inputs(inputs):
    import ml_dtypes
    bf = ml_dtypes.bfloat16
    x = np.asarray(inputs["x"], dtype=np.float32)         # [B, T, N, D]
    # fm layout: [B, (j=t%2, d), tp, n]
    x_fm = np.ascontiguousarray(
        x.reshape(B, TP, 2, N, D).transpose(0, 2, 4, 1, 3)
        .reshape(B, 128, TP, N)).astype(bf)
    sup = np.asarray(inputs["supports"], dtype=np.float32)
    a_bf = np.ascontiguousarray(sup).astype(bf)
    at_bf = np.ascontiguousarray(sup.transpose(0, 2, 1)).astype(bf)
    bqkv = np.stack([np.asarray(inputs["bq"], np.float32),
                     np.asarray(inputs["bk"], np.float32),
                     np.asarray(inputs["bv"], np.float32)], axis=1)  # [L,3,D]
    shared = {"a_bf": a_bf, "at_bf": at_bf,
              "bqkv": np.ascontiguousarray(bqkv)}
    names = ["Wg", "bg", "Wq", "Wk", "Wv", "Wo", "bo", "W1", "b1", "W2", "b2",
             "ln1_g", "ln1_b", "ln2_g", "ln2_b", "Wout", "bout"]
    for n in names:
        shared[n] = np.ascontiguousarray(np.asarray(inputs[n], dtype=np.float32))
    qkv_bias = bool(np.any(bqkv))
    in_maps = []
    for c in range(NCORES):
        m = dict(shared)
        m["x"] = np.ascontiguousarray(x_fm[c * BL:(c + 1) * BL])
        in_maps.append(m)
    return in_maps, qkv_bias


def kernel(**inputs):
    from concourse.bass_utils import run_bass_kernel_spmd
    in_maps, qkv_bias = _prep_inputs(inputs)
    nc, taps = _get_nc(qkv_bias)
    res = run_bass_kernel_spmd(nc, in_maps, core_ids=list(range(NCORES)))
    _CACHE["last_res"] = res
    out = np.concatenate([r["out"] for r in res.results], axis=0)
    return out.astype(np.float32)

